# revision 37
# baseline (speedup 1.0000x reference)
"""Distributed attention kernel for TRN2 (8 NeuronCores).

Computes: softmax(sqrt(Dqk) * (x@Wq.T) @ (x@Wk.T).T) @ (x@Wv.T)
for x [8192, 1024], Wq/Wk/Wv [256, 1024], out [8192, 256].

Sharding: rows of x across 8 cores (sequence parallel). Weights replicated.
Each core projects its shard, AllGathers K^T (f16) + V (bf16) in ONE packed
collective, then runs flash-style attention over its 1024 Q rows.

v2 structure (vs v1 baseline at ~298us):
  - q^T/k^T are f16 (measured rel-err impact ~3e-3, gate is 2e-2): halves
    the AG payload (1.57MB -> 1MB), kt_full SBUF (64 -> 32KB/lane) and
    assembly bytes.
  - AG fires as early as possible; phase A emits only the k/v-projection
    path before the trigger (x^T, Wk/Wv^T, k proj, staging, v proj, pack).
    Wq^T + q proj + the ENTIRE ch0 (local) chunk pipeline (scores, exp,
    P^T, PV) overlap the ~50us collective.
  - ch0 P^T transposes go on the SCALAR queue (the sync queue holds the
    anchor + V assembly DMAs that wait on the AG semaphore; anything
    behind them would stall). ch>=1 P^T transposes alternate sync/scalar.
  - assembly: V on sync (corrupts on other queues - empirical, from v1),
    kt (one DMA per slot) on scalar.
  - chunk row-max on GpSimd/Pool (frees DVE), exp bias = RUNNING max, so
    the per-chunk beta correction disappears and the accumulator merge is
    a single scalar_tensor_tensor (acc = acc*gamma + po) on DVE.
  - flash stats (alpha, sig) keyed to the running max nmr.

Per 128-row i-tile main loop: scores chunks in PSUM (f16 matmul), chunk
row-max on Pool, exp on ACT (PSUM -> SBUF bf16, running-max bias, row-sums
via accum_out), P^T via DMA xbar transpose, PV matmul bf16, single-op
accumulator merge, normalize, DMA out.
"""

import numpy as np

import concourse.bacc as bacc
import concourse.bass as bass
import concourse.mybir as mybir
import concourse.tile as tile
from concourse.bass_utils import run_bass_kernel_spmd
from concourse.masks import make_identity

F32 = mybir.dt.float32
F32R = mybir.dt.float32r
BF16 = mybir.dt.bfloat16
F16 = mybir.dt.float16

N_CORES = 8
N, D, DQK, DV = 8192, 1024, 256, 256
P = 128
CHUNK = 1024  # scores chunk width (2 PSUM banks)


def build(n=N, d=D, dqk=DQK, dv=DV, ncores=N_CORES):
    nsh = n // ncores
    IT = nsh // P       # i-tiles per core
    KT = d // P         # contract tiles for projections
    CT = dqk // P       # dqk tiles
    JT = n // P         # j tiles for PV
    chunk = min(CHUNK, nsh)
    assert chunk == nsh, "rotation requires one chunk per rank"
    NCH = n // chunk    # score chunks per row == ncores
    W512 = min(512, chunk)
    NN = chunk // W512  # W512-wide matmuls per chunk
    scale = float(np.sqrt(dqk))

    # Two packed AG buffers, rows of nsh u16 (2KB):
    #   pack_k: k^T as [dqk, nsh] f16 — AllGathered FIRST (gates scores)
    #   pack_v: v as [nsh, dv] bf16; partition p's [IT, dv] slab =
    #           rows 2p, 2p+1 — AllGathered second (gates PV only)
    U16 = mybir.dt.uint16

    nc = bacc.Bacc(None, target_bir_lowering=False, num_devices=ncores)

    x_ext = nc.declare_dram_parameter("x", [nsh, d], F32, isOutput=False)
    wq_ext = nc.declare_dram_parameter("Wq", [dqk, d], F32, isOutput=False)
    wk_ext = nc.declare_dram_parameter("Wk", [dqk, d], F32, isOutput=False)
    wv_ext = nc.declare_dram_parameter("Wv", [dv, d], F32, isOutput=False)
    out_ext = nc.declare_dram_parameter("out", [nsh, dv], F32, isOutput=True)

    groups = [list(range(ncores))]

    with tile.TileContext(nc) as tc:
        with (
            tc.tile_pool(name="persist", bufs=1) as pp,
            tc.tile_pool(name="dramp", bufs=1, space="DRAM") as dp,
        ):
            pack_k = dp.tile([dqk, nsh], U16, name="pack_k")
            pack_k_ag = dp.tile(
                [ncores * dqk, nsh], U16, addr_space="Shared",
                name="pack_k_ag",
            )
            pack_v = dp.tile([2 * P, nsh], U16, name="pack_v")
            pack_v_ag = dp.tile(
                [ncores * 2 * P, nsh], U16, addr_space="Shared",
                name="pack_v_ag",
            )
            dvp = dv + 1  # extra ones-column: PV accumulates the
            # softmax denominator alongside the numerator (gamma
            # corrections apply to both for free)
            qt_s = pp.tile([P, CT, nsh], F16, tag="qt")
            kt_loc = pp.tile([P, CT, nsh], F16, tag="ktloc")
            v_loc = pp.tile([P, IT, dvp], BF16, tag="vloc")
            nc.vector.memset(v_loc[:, :, dv:dvp], 1.0)
            ident = pp.tile([P, P], F32, tag="ident")
            make_identity(nc, ident[:])
            ident_b = pp.tile([P, P], BF16, tag="identb")
            make_identity(nc, ident_b[:])

            # ================= Phase A =================
            with (
                tc.tile_pool(name="phA", bufs=1) as pa,
                tc.tile_pool(name="phA_psum", bufs=1, space="PSUM") as paps,
            ):
                x_nat = pa.tile([P, IT, d], F32, tag="xnat")
                w_nat = pa.tile([P, 3 * CT, d], F32, tag="wnat")
                w_exts = {0: wq_ext, 1: wk_ext, 2: wv_ext}

                def w_dma(wi):
                    nc.scalar.dma_start(
                        w_nat[:, wi * CT:(wi + 1) * CT, :],
                        w_exts[wi].ap().rearrange("(ct p) d -> p ct d", p=P),
                    )

                def x_dma(it, eng):
                    eng.dma_start(
                        x_nat[:, it, :],
                        x_ext.ap().rearrange("(it p) d -> p it d", p=P)[:, it, :],
                    )

                # scalar queue: Wk FIRST (it gates the Wk transposes ->
                # k-proj -> AG1 path); x evens go on sync in parallel
                w_dma(1)
                for it in range(0, IT, 2):
                    x_dma(it, nc.sync)
                for it in range(1, IT, 2):
                    x_dma(it, nc.scalar)
                w_dma(2)
                w_dma(0)

                xt_s = pa.tile([P, KT, nsh], F32R, tag="xt")
                wt_s = pa.tile([P, 3 * KT, dqk], F32R, tag="wt")

                ei = 0  # eviction engine alternator

                def evict(dst, src):
                    nonlocal ei
                    ei += 1
                    if ei % 2:
                        nc.vector.tensor_copy(dst, src)
                    else:
                        nc.scalar.copy(dst, src)

                # interleave x^T with the k-projection path so k staging
                # (which gates the AllGather) starts as early as possible
                ich_w = min(512, nsh)

                def x_transposes(it_range):
                    for it in it_range:
                        for kt in range(KT):
                            tp = paps.tile([P, P], F32, tag="tp", bufs=4)
                            nc.tensor.transpose(
                                tp[:], x_nat[:, it, kt * P:(kt + 1) * P], ident[:]
                            )
                            evict(xt_s[:, kt, it * P:(it + 1) * P], tp[:])

                def w_transposes(wi):
                    for kt in range(KT):
                        for ct in range(CT):
                            tp = paps.tile([P, P], F32, tag="tp", bufs=4)
                            nc.tensor.transpose(
                                tp[:],
                                w_nat[:, wi * CT + ct, kt * P:(kt + 1) * P],
                                ident[:],
                            )
                            evict(wt_s[:, wi * KT + kt, ct * P:(ct + 1) * P], tp[:])

                def k_proj(ich):
                    for ct in range(CT):
                        psk = paps.tile([P, ich_w], F32, tag="pqk", bufs=2)
                        for kt in range(KT):
                            nc.tensor.matmul(
                                psk[:],
                                wt_s[:, 1 * KT + kt, ct * P:(ct + 1) * P],
                                xt_s[:, kt, ich * ich_w:(ich + 1) * ich_w],
                                start=(kt == 0),
                                stop=(kt == KT - 1),
                            )
                        nc.vector.tensor_copy(
                            kt_loc[:, ct, ich * ich_w:(ich + 1) * ich_w],
                            psk[:],
                        )
                    # stage this ich half immediately (AG waits on staging)
                    nc.sync.dma_start(
                        pack_k[:].bitcast(F16).rearrange(
                            "(ct p) i -> p ct i", p=P
                        )[:, :, ich * ich_w:(ich + 1) * ich_w],
                        kt_loc[:, :, ich * ich_w:(ich + 1) * ich_w],
                    )

                # k_proj(ich) only needs x^T of i-rows in that half:
                # interleave so AG1 triggers as early as possible
                ipw = ich_w // P
                x_transposes(range(ipw))
                w_transposes(1)           # Wk
                k_proj(0)
                x_transposes(range(ipw, IT))
                k_proj(1)
                # K AllGather fires as soon as k^T is staged; scores for
                # ch>=1 depend only on this one
                nc.gpsimd.collective_compute(
                    "AllGather",
                    mybir.AluOpType.bypass,
                    replica_groups=groups,
                    ins=[pack_k[:].opt()],
                    outs=[pack_k_ag[:].opt()],
                )
                w_transposes(2)           # Wv
                # v projections
                for it in range(IT):
                    psv = paps.tile([P, dv], F32, tag="psv", bufs=2)
                    for kt in range(KT):
                        nc.tensor.matmul(
                            psv[:],
                            xt_s[:, kt, it * P:(it + 1) * P],
                            wt_s[:, 2 * KT + kt, :dqk],
                            start=(kt == 0),
                            stop=(kt == KT - 1),
                        )
                    nc.vector.tensor_copy(v_loc[:, it, :dv], psv[:])
                # v packed: partition p's [IT, dv] slab -> rows 2p, 2p+1
                # (the ones column stays local; receivers memset their own)
                nc.sync.dma_start(
                    pack_v[:].bitcast(BF16).rearrange(
                        "(p r) i -> p (r i)", p=P
                    ),
                    v_loc[:, :, :dv],
                )
                # V AllGather (runs after the K one on the CC stream)
                nc.gpsimd.collective_compute(
                    "AllGather",
                    mybir.AluOpType.bypass,
                    replica_groups=groups,
                    ins=[pack_v[:].opt()],
                    outs=[pack_v_ag[:].opt()],
                )

                # Wq transposes + q^T projection (overlap the AllGather).
                # ich outer so q rows 0-511 finish first and ch0 scores
                # can start while the second half projects.
                w_transposes(0)
                for ich in range(nsh // ich_w):
                    for ct in range(CT):
                        psq = paps.tile([P, ich_w], F32, tag="pqk", bufs=2)
                        for kt in range(KT):
                            nc.tensor.matmul(
                                psq[:],
                                wt_s[:, 0 * KT + kt, ct * P:(ct + 1) * P],
                                xt_s[:, kt, ich * ich_w:(ich + 1) * ich_w],
                                start=(kt == 0),
                                stop=(kt == KT - 1),
                            )
                        nc.vector.tensor_scalar_mul(
                            qt_s[:, ct, ich * ich_w:(ich + 1) * ich_w],
                            psq[:],
                            scale,
                        )

            phb_cm = tc.tile_pool(name="phB", bufs=1)
            phb = phb_cm.__enter__()
            kt_full = phb.tile([P, CT, n], F16, tag="ktf", name="kt_full")
            v_s = phb.tile([P, JT, dvp], BF16, tag="vs", name="v_s")
            nc.vector.memset(v_s[:, :, dv:dvp], 1.0)

            pag_h = pack_k_ag[:].bitcast(F16)
            pag_b = pack_v_ag[:].bitcast(BF16)

            def assemble_v(s, rk_sync):
                # slot s holds rank (my_rank + s) % ncores; V must be on
                # the sync queue (corrupts on any other - empirical)
                vrow = ((rk_sync + s) % ncores) * 2 * P
                nc.sync.dma_start(
                    v_s[:, s * IT:(s + 1) * IT, :dv],
                    pag_b[bass.ds(vrow, 2 * P), :].rearrange(
                        "(p r) i -> p (r i)", p=P
                    ),
                )

            def assemble_kt(s, rk_kt):
                row = ((rk_kt + s) % ncores) * dqk
                nc.sync.dma_start(
                    kt_full[:, :, s * nsh:(s + 1) * nsh],
                    pag_h[bass.ds(row, dqk), :].rearrange(
                        "(ct p) i -> p ct i", p=P
                    ),
                )

            # ================= Main attention loop =================
            # Streaming flash-attention: each 1024-wide chunk flows
            # MM -> row-max(Pool) -> exp(ACT, running-max bias) -> P^T
            # transpose -> PV -> one-op accumulator merge.
            with (
                tc.tile_pool(name="mainA", bufs=8) as ma,
                tc.tile_pool(name="chunkp", bufs=2) as cp,
                tc.tile_pool(name="scores_psum", bufs=3, space="PSUM") as sps,
                tc.tile_pool(name="out_psum", bufs=2, space="PSUM") as ops,
            ):
                JPC = chunk // P  # j-tiles per chunk (== IT)

                NTOT = IT * NCH
                stats = []
                for it in range(IT):
                    st = {
                        "mneg": ma.tile([P, NCH], F32, tag="mneg", name="mneg"),
                        "nmr": ma.tile([P, NCH], F32, tag="nmr", name="nmr"),
                        "rinv": ma.tile([P, 1], F32, tag="rinv", name="rinv"),
                        "gam": ma.tile([P, NCH], F32, tag="gam", name="gam"),
                        "acc": ma.tile([P, dvp], F32, tag="acc", name="acc"),
                    }
                    stats.append(st)

                import collections
                pend = collections.deque()  # (k, pt_c) with deep PV lag

                def do_scores(k):
                    ch, it = divmod(k, IT)
                    st = stats[it]
                    ps = sps.tile([P, chunk], F32, tag="s", name="ps")
                    for ct in range(CT):
                        for nn in range(NN):
                            nc.tensor.matmul(
                                ps[:, nn * W512:(nn + 1) * W512],
                                qt_s[:, ct, it * P:(it + 1) * P],
                                (kt_loc[:, ct, nn * W512:(nn + 1) * W512]
                                 if ch == 0 else
                                 kt_full[
                                     :, ct,
                                     ch * chunk + nn * W512:
                                     ch * chunk + (nn + 1) * W512,
                                 ]),
                                start=(ct == 0),
                                stop=(ct == CT - 1),
                                skip_group_check=True,
                            )
                    nc.vector.reduce_max(
                        st["mneg"][:, ch:ch + 1], ps[:],
                        axis=mybir.AxisListType.X, negate=True,
                    )
                    if ch > 0:
                        nc.vector.tensor_tensor(
                            st["nmr"][:, ch:ch + 1], st["nmr"][:, ch - 1:ch],
                            st["mneg"][:, ch:ch + 1], op=mybir.AluOpType.min,
                        )
                    else:
                        nc.vector.tensor_copy(st["nmr"][:, :1], st["mneg"][:, :1])
                    p_c = cp.tile([P, chunk], BF16, tag="p", name="p_c", bufs=16)
                    # bias = running max -> P is exp(s - m_run(ch)); no
                    # beta, and no accum_out: the denominator comes from
                    # the PV ones-column
                    nc.scalar.activation(
                        p_c[:], ps[:],
                        mybir.ActivationFunctionType.Exp,
                        bias=st["nmr"][:, ch:ch + 1],
                        scale=1.0,
                    )
                    if ch < 2:
                        # xbar transposes are serialized against in-flight
                        # collectives; ch0/ch1 (which overlap the K and V
                        # AllGathers) must transpose on the PE instead
                        return p_c
                    pt_c = cp.tile([P, JPC, P], BF16, tag="pt", name="pt_c", bufs=30)
                    # DMA_TRANSPOSE burns ~1.3us of ISSUING-ENGINE time;
                    # keep them all on the otherwise-idle sync engine
                    nc.sync.dma_start_transpose(pt_c[:], p_c[:])
                    return pt_c

                def do_tr_pe(p_c_t):
                    # P^T via PE for ch0 (PSUM is full: borrow a scores-
                    # ring tile and pack the 8 bf16 transpose outputs into
                    # its first half via bitcast slices)
                    pt_c = cp.tile([P, JPC, P], BF16, tag="pt", name="pt_c", bufs=30)
                    ps_tr = sps.tile([P, chunk], F32, tag="s", name="ps_tr")
                    for j2 in range(JPC):
                        tpp = ps_tr[:, j2 * 64:(j2 + 1) * 64].bitcast(BF16)
                        nc.tensor.transpose(
                            tpp, p_c_t[:, j2 * P:(j2 + 1) * P], ident_b[:]
                        )
                        if j2 % 2:
                            nc.vector.tensor_copy(pt_c[:, j2, :], tpp)
                        else:
                            nc.scalar.copy(pt_c[:, j2, :], tpp)
                    return pt_c

                def do_pv(k, pt_c):
                    ch, it = divmod(k, IT)
                    st = stats[it]
                    po = ops.tile([P, dvp], F32, tag="po", name="po")
                    for j2 in range(JPC):
                        nc.tensor.matmul(
                            po[:], pt_c[:, j2, :],
                            (v_loc[:, j2, :] if ch == 0 else
                             v_s[:, ch * JPC + j2, :]),
                            start=(j2 == 0), stop=(j2 == JPC - 1),
                        )
                    if ch == 0:
                        nc.vector.tensor_copy(st["acc"][:], po[:])
                    else:
                        # gamma = exp(m_run(ch-1) - m_run(ch))
                        nc.scalar.activation(
                            st["gam"][:, ch:ch + 1], st["nmr"][:, ch - 1:ch],
                            mybir.ActivationFunctionType.Exp,
                            bias=st["nmr"][:, ch:ch + 1], scale=-1.0,
                        )
                        # acc = acc*gamma + po (P already carries beta via
                        # the running-max exp bias)
                        nc.vector.scalar_tensor_tensor(
                            st["acc"][:], st["acc"][:], st["gam"][:, ch:ch + 1],
                            po[:],
                            op0=mybir.AluOpType.mult,
                            op1=mybir.AluOpType.add,
                        )
                    if ch == NCH - 1:
                        # denominator rode along in the ones-column
                        nc.vector.reciprocal(
                            st["rinv"][:], st["acc"][:, dv:dvp]
                        )
                        nc.vector.tensor_scalar_mul(
                            st["acc"][:, :dv], st["acc"][:, :dv], st["rinv"][:]
                        )
                        nc.sync.dma_start(
                            out_ext.ap().rearrange("(it p) c -> p it c", p=P)[
                                :, it, :
                            ],
                            st["acc"][:, :dv],
                        )

                order = [g * IT + i2 for g in range(NCH) for i2 in range(IT)]
                LAG = min(28, max(1, len(order) - 1))
                LAG0 = 3  # shallow lag inside ch0 so it completes in-AG
                anchor = ma.tile([1, 64], U16, tag="anchor", name="anchor",
                                 bufs=2)

                # NOTE: Tile rotates hardware DMAs over 8 completion
                # semaphores in EMISSION order; a DMA must wait for its
                # lane's previous occupant. AG-gated assembly DMAs must
                # therefore be emitted AFTER all of ch0's transposes, or
                # ch0 (which is AG-independent) transitively waits on the
                # collective.
                TRPE = 2 * IT  # chunks with PE transposes (AG overlap)
                sc_pend = collections.deque()  # (k, p_c) awaiting PE tr
                for k in order:
                    if k == IT:
                        # ch0 fully emitted; flush its transposes + PVs
                        # ahead of the first AG1-dependent scores (ch1
                        # PVs stay queued: they need the V AllGather)
                        while sc_pend:
                            kk, pc_t = sc_pend.popleft()
                            pend.append((kk, do_tr_pe(pc_t)))
                        while pend:
                            kk, pt = pend.popleft()
                            do_pv(kk, pt)
                        # kt assembly first (scores ch1 needs slot 1
                        # before PV needs V). anchor = static-offset
                        # pack_ag read carrying the collective wait for
                        # the dynamic-offset DMAs, which Tile can't track.
                        # tile_wait_until keeps the scheduler from
                        # hoisting these AG-gated DMAs ahead of ch0's
                        # transposes/exps in the engine streams (which
                        # parks those engines on the collective).
                        # everything on the SYNC engine: it is idle at
                        # AG1-completion (exps/evictions keep scalar busy
                        # for ~30us more), so assembly starts immediately
                        with tc.tile_wait_until(1.0):
                            nc.sync.dma_start(
                                anchor[:1, :], pack_k_ag[:][1:2, 0:64]
                            )
                            rk_sync = nc.sync.cc_rank(groups)
                            for s in range(1, NCH):
                                assemble_kt(s, rk_sync)
                            nc.sync.dma_start(
                                anchor[:1, :], pack_v_ag[:][0:1, 0:64]
                            )
                            for s in range(1, NCH):
                                assemble_v(s, rk_sync)
                    if k == TRPE:
                        # last PE-transposed chunk flushes before the
                        # first xbar-transposed one
                        while sc_pend:
                            kk, pc_t = sc_pend.popleft()
                            pend.append((kk, do_tr_pe(pc_t)))
                    res = do_scores(k)
                    if k < TRPE:
                        # PE-transpose lags scores by one chunk so the
                        # in-order PE never stalls on the exp chain
                        sc_pend.append((k, res))
                        if len(sc_pend) > 1:
                            kk, pc_t = sc_pend.popleft()
                            pend.append((kk, do_tr_pe(pc_t)))
                    else:
                        pend.append((k, res))
                    lag = LAG0 if k < IT else LAG
                    if len(pend) > lag:
                        kk, pt = pend.popleft()
                        do_pv(kk, pt)
                while pend:
                    kk, pt = pend.popleft()
                    do_pv(kk, pt)

            phb_cm.__exit__(None, None, None)

    nc.finalize()
    return nc


_NC_CACHE = {}


def _get_nc(key):
    if key not in _NC_CACHE:
        n, d, dqk, dv, ncores = key
        _NC_CACHE[key] = build(n=n, d=d, dqk=dqk, dv=dv, ncores=ncores)
    return _NC_CACHE[key]


def run(x, Wq, Wk, Wv, trace=False):
    n, d = x.shape
    dqk = Wq.shape[0]
    dv = Wv.shape[0]
    ncores = N_CORES
    nsh = n // ncores
    nc = _get_nc((n, d, dqk, dv, ncores))

    x = np.ascontiguousarray(x, dtype=np.float32)
    Wq = np.ascontiguousarray(Wq, dtype=np.float32)
    Wk = np.ascontiguousarray(Wk, dtype=np.float32)
    Wv = np.ascontiguousarray(Wv, dtype=np.float32)

    in_maps = [
        {"x": x[r * nsh:(r + 1) * nsh], "Wq": Wq, "Wk": Wk, "Wv": Wv}
        for r in range(ncores)
    ]
    res = run_bass_kernel_spmd(
        nc, in_maps, core_ids=list(range(ncores)), trace=trace
    )
    out = np.concatenate([res.results[r]["out"] for r in range(ncores)], axis=0)
    return out, res


def kernel(x, Wq, Wk, Wv):
    out, _ = run(x, Wq, Wk, Wv)
    return out


# revision 38
# speedup vs baseline: 1.0321x; 1.0321x over previous
"""Distributed attention kernel for TRN2 (8 NeuronCores).

Computes: softmax(sqrt(Dqk) * (x@Wq.T) @ (x@Wk.T).T) @ (x@Wv.T)
for x [8192, 1024], Wq/Wk/Wv [256, 1024], out [8192, 256].

Sharding: rows of x across 8 cores (sequence parallel). Weights replicated.
Each core projects its shard, AllGathers K^T (f16) + V (bf16) in ONE packed
collective, then runs flash-style attention over its 1024 Q rows.

v2 structure (vs v1 baseline at ~298us):
  - q^T/k^T are f16 (measured rel-err impact ~3e-3, gate is 2e-2): halves
    the AG payload (1.57MB -> 1MB), kt_full SBUF (64 -> 32KB/lane) and
    assembly bytes.
  - AG fires as early as possible; phase A emits only the k/v-projection
    path before the trigger (x^T, Wk/Wv^T, k proj, staging, v proj, pack).
    Wq^T + q proj + the ENTIRE ch0 (local) chunk pipeline (scores, exp,
    P^T, PV) overlap the ~50us collective.
  - ch0 P^T transposes go on the SCALAR queue (the sync queue holds the
    anchor + V assembly DMAs that wait on the AG semaphore; anything
    behind them would stall). ch>=1 P^T transposes alternate sync/scalar.
  - assembly: V on sync (corrupts on other queues - empirical, from v1),
    kt (one DMA per slot) on scalar.
  - chunk row-max on GpSimd/Pool (frees DVE), exp bias = RUNNING max, so
    the per-chunk beta correction disappears and the accumulator merge is
    a single scalar_tensor_tensor (acc = acc*gamma + po) on DVE.
  - flash stats (alpha, sig) keyed to the running max nmr.

Per 128-row i-tile main loop: scores chunks in PSUM (f16 matmul), chunk
row-max on Pool, exp on ACT (PSUM -> SBUF bf16, running-max bias, row-sums
via accum_out), P^T via DMA xbar transpose, PV matmul bf16, single-op
accumulator merge, normalize, DMA out.
"""

import numpy as np

import concourse.bacc as bacc
import concourse.bass as bass
import concourse.mybir as mybir
import concourse.tile as tile
from concourse.bass_utils import run_bass_kernel_spmd
from concourse.masks import make_identity

F32 = mybir.dt.float32
F32R = mybir.dt.float32r
BF16 = mybir.dt.bfloat16
F16 = mybir.dt.float16

N_CORES = 8
N, D, DQK, DV = 8192, 1024, 256, 256
P = 128
CHUNK = 1024  # scores chunk width (2 PSUM banks)


def build(n=N, d=D, dqk=DQK, dv=DV, ncores=N_CORES):
    nsh = n // ncores
    IT = nsh // P       # i-tiles per core
    KT = d // P         # contract tiles for projections
    CT = dqk // P       # dqk tiles
    JT = n // P         # j tiles for PV
    chunk = min(CHUNK, nsh)
    assert chunk == nsh, "rotation requires one chunk per rank"
    NCH = n // chunk    # score chunks per row == ncores
    W512 = min(512, chunk)
    NN = chunk // W512  # W512-wide matmuls per chunk
    scale = float(np.sqrt(dqk))

    # Two packed AG buffers, rows of nsh u16 (2KB):
    #   pack_k: k^T as [dqk, nsh] f16 — AllGathered FIRST (gates scores)
    #   pack_v: v as [nsh, dv] bf16; partition p's [IT, dv] slab =
    #           rows 2p, 2p+1 — AllGathered second (gates PV only)
    U16 = mybir.dt.uint16

    nc = bacc.Bacc(None, target_bir_lowering=False, num_devices=ncores)

    x_ext = nc.declare_dram_parameter("x", [nsh, d], F32, isOutput=False)
    wq_ext = nc.declare_dram_parameter("Wq", [dqk, d], F32, isOutput=False)
    wk_ext = nc.declare_dram_parameter("Wk", [dqk, d], F32, isOutput=False)
    wv_ext = nc.declare_dram_parameter("Wv", [dv, d], F32, isOutput=False)
    out_ext = nc.declare_dram_parameter("out", [nsh, dv], F32, isOutput=True)

    groups = [list(range(ncores))]

    with tile.TileContext(nc) as tc:
        with (
            tc.tile_pool(name="persist", bufs=1) as pp,
            tc.tile_pool(name="dramp", bufs=1, space="DRAM") as dp,
        ):
            pack_k = dp.tile([dqk, nsh], U16, name="pack_k")
            pack_k_ag = dp.tile(
                [ncores * dqk, nsh], U16, addr_space="Shared",
                name="pack_k_ag",
            )
            pack_v = dp.tile([2 * P, nsh], U16, name="pack_v")
            pack_v_ag = dp.tile(
                [ncores * 2 * P, nsh], U16, addr_space="Shared",
                name="pack_v_ag",
            )
            dvp = dv + 1  # extra ones-column: PV accumulates the
            # softmax denominator alongside the numerator (gamma
            # corrections apply to both for free)
            qt_s = pp.tile([P, CT, nsh], F16, tag="qt")
            kt_loc = pp.tile([P, CT, nsh], F16, tag="ktloc")
            v_loc = pp.tile([P, IT, dvp], BF16, tag="vloc")
            nc.vector.memset(v_loc[:, :, dv:dvp], 1.0)
            ident = pp.tile([P, P], F32, tag="ident")
            make_identity(nc, ident[:])
            ident_b = pp.tile([P, P], BF16, tag="identb")
            make_identity(nc, ident_b[:])

            # ================= Phase A =================
            with (
                tc.tile_pool(name="phA", bufs=1) as pa,
                tc.tile_pool(name="phA_psum", bufs=1, space="PSUM") as paps,
            ):
                x_nat = pa.tile([P, IT, d], F32, tag="xnat")
                w_nat = pa.tile([P, 3 * CT, d], F32, tag="wnat")
                w_exts = {0: wq_ext, 1: wk_ext, 2: wv_ext}

                def w_dma(wi):
                    nc.scalar.dma_start(
                        w_nat[:, wi * CT:(wi + 1) * CT, :],
                        w_exts[wi].ap().rearrange("(ct p) d -> p ct d", p=P),
                    )

                def x_dma(it, eng):
                    eng.dma_start(
                        x_nat[:, it, :],
                        x_ext.ap().rearrange("(it p) d -> p it d", p=P)[:, it, :],
                    )

                # scalar queue: Wk FIRST (it gates the Wk transposes ->
                # k-proj -> AG1 path); x evens go on sync in parallel
                w_dma(1)
                for it in range(0, IT, 2):
                    x_dma(it, nc.sync)
                for it in range(1, IT, 2):
                    x_dma(it, nc.scalar)
                w_dma(2)
                w_dma(0)

                xt_s = pa.tile([P, KT, nsh], F32R, tag="xt")
                wt_s = pa.tile([P, 3 * KT, dqk], F32R, tag="wt")

                ei = 0  # eviction engine alternator

                def evict(dst, src):
                    nonlocal ei
                    ei += 1
                    if ei % 2:
                        nc.vector.tensor_copy(dst, src)
                    else:
                        nc.scalar.copy(dst, src)

                # interleave x^T with the k-projection path so k staging
                # (which gates the AllGather) starts as early as possible
                ich_w = min(512, nsh)

                def x_transposes(it_range):
                    for it in it_range:
                        for kt in range(KT):
                            tp = paps.tile([P, P], F32, tag="tp", bufs=4)
                            nc.tensor.transpose(
                                tp[:], x_nat[:, it, kt * P:(kt + 1) * P], ident[:]
                            )
                            evict(xt_s[:, kt, it * P:(it + 1) * P], tp[:])

                def w_transposes(wi):
                    for kt in range(KT):
                        for ct in range(CT):
                            tp = paps.tile([P, P], F32, tag="tp", bufs=4)
                            nc.tensor.transpose(
                                tp[:],
                                w_nat[:, wi * CT + ct, kt * P:(kt + 1) * P],
                                ident[:],
                            )
                            evict(wt_s[:, wi * KT + kt, ct * P:(ct + 1) * P], tp[:])

                def k_proj(ich):
                    for ct in range(CT):
                        psk = paps.tile([P, ich_w], F32, tag="pqk", bufs=2)
                        for kt in range(KT):
                            nc.tensor.matmul(
                                psk[:],
                                wt_s[:, 1 * KT + kt, ct * P:(ct + 1) * P],
                                xt_s[:, kt, ich * ich_w:(ich + 1) * ich_w],
                                start=(kt == 0),
                                stop=(kt == KT - 1),
                            )
                        nc.vector.tensor_copy(
                            kt_loc[:, ct, ich * ich_w:(ich + 1) * ich_w],
                            psk[:],
                        )
                    # stage this ich half immediately (AG waits on staging)
                    nc.sync.dma_start(
                        pack_k[:].bitcast(F16).rearrange(
                            "(ct p) i -> p ct i", p=P
                        )[:, :, ich * ich_w:(ich + 1) * ich_w],
                        kt_loc[:, :, ich * ich_w:(ich + 1) * ich_w],
                    )

                # k_proj(ich) only needs x^T of i-rows in that half:
                # interleave so AG1 triggers as early as possible
                ipw = ich_w // P
                x_transposes(range(ipw))
                w_transposes(1)           # Wk
                k_proj(0)
                x_transposes(range(ipw, IT))
                k_proj(1)
                # K AllGather fires as soon as k^T is staged; scores for
                # ch>=1 depend only on this one
                nc.gpsimd.collective_compute(
                    "AllGather",
                    mybir.AluOpType.bypass,
                    replica_groups=groups,
                    ins=[pack_k[:].opt()],
                    outs=[pack_k_ag[:].opt()],
                )
                w_transposes(2)           # Wv
                # v projections
                for it in range(IT):
                    psv = paps.tile([P, dv], F32, tag="psv", bufs=2)
                    for kt in range(KT):
                        nc.tensor.matmul(
                            psv[:],
                            xt_s[:, kt, it * P:(it + 1) * P],
                            wt_s[:, 2 * KT + kt, :dqk],
                            start=(kt == 0),
                            stop=(kt == KT - 1),
                        )
                    nc.vector.tensor_copy(v_loc[:, it, :dv], psv[:])
                # v packed: partition p's [IT, dv] slab -> rows 2p, 2p+1
                # (the ones column stays local; receivers memset their own)
                nc.sync.dma_start(
                    pack_v[:].bitcast(BF16).rearrange(
                        "(p r) i -> p (r i)", p=P
                    ),
                    v_loc[:, :, :dv],
                )
                # V AllGather (runs after the K one on the CC stream)
                nc.gpsimd.collective_compute(
                    "AllGather",
                    mybir.AluOpType.bypass,
                    replica_groups=groups,
                    ins=[pack_v[:].opt()],
                    outs=[pack_v_ag[:].opt()],
                )

                # Wq transposes + q^T projection (overlap the AllGather).
                # ich outer so q rows 0-511 finish first and ch0 scores
                # can start while the second half projects.
                w_transposes(0)
                for ich in range(nsh // ich_w):
                    for ct in range(CT):
                        psq = paps.tile([P, ich_w], F32, tag="pqk", bufs=2)
                        for kt in range(KT):
                            nc.tensor.matmul(
                                psq[:],
                                wt_s[:, 0 * KT + kt, ct * P:(ct + 1) * P],
                                xt_s[:, kt, ich * ich_w:(ich + 1) * ich_w],
                                start=(kt == 0),
                                stop=(kt == KT - 1),
                            )
                        nc.vector.tensor_scalar_mul(
                            qt_s[:, ct, ich * ich_w:(ich + 1) * ich_w],
                            psq[:],
                            scale,
                        )

            phb_cm = tc.tile_pool(name="phB", bufs=1)
            phb = phb_cm.__enter__()
            kt_full = phb.tile([P, CT, n], F16, tag="ktf", name="kt_full")
            v_s = phb.tile([P, JT, dvp], BF16, tag="vs", name="v_s")
            nc.vector.memset(v_s[:, :, dv:dvp], 1.0)

            pag_h = pack_k_ag[:].bitcast(F16)
            pag_b = pack_v_ag[:].bitcast(BF16)

            def assemble_v(s, rk_sync):
                # slot s holds rank (my_rank + s) % ncores; V must be on
                # the sync queue (corrupts on any other - empirical)
                vrow = ((rk_sync + s) % ncores) * 2 * P
                nc.sync.dma_start(
                    v_s[:, s * IT:(s + 1) * IT, :dv],
                    pag_b[bass.ds(vrow, 2 * P), :].rearrange(
                        "(p r) i -> p (r i)", p=P
                    ),
                )

            def assemble_kt(s, rk_kt):
                row = ((rk_kt + s) % ncores) * dqk
                nc.sync.dma_start(
                    kt_full[:, :, s * nsh:(s + 1) * nsh],
                    pag_h[bass.ds(row, dqk), :].rearrange(
                        "(ct p) i -> p ct i", p=P
                    ),
                )

            # ================= Main attention loop =================
            # Streaming flash-attention: each 1024-wide chunk flows
            # MM -> row-max(Pool) -> exp(ACT, running-max bias) -> P^T
            # transpose -> PV -> one-op accumulator merge.
            with (
                tc.tile_pool(name="mainA", bufs=8) as ma,
                tc.tile_pool(name="chunkp", bufs=2) as cp,
                tc.tile_pool(name="scores_psum", bufs=3, space="PSUM") as sps,
                tc.tile_pool(name="out_psum", bufs=2, space="PSUM") as ops,
            ):
                JPC = chunk // P  # j-tiles per chunk (== IT)

                NTOT = IT * NCH
                stats = []
                for it in range(IT):
                    st = {
                        "mneg": ma.tile([P, NCH], F32, tag="mneg", name="mneg"),
                        "nmr": ma.tile([P, NCH], F32, tag="nmr", name="nmr"),
                        "rinv": ma.tile([P, 1], F32, tag="rinv", name="rinv"),
                        "gam": ma.tile([P, NCH], F32, tag="gam", name="gam"),
                        "acc": ma.tile([P, dvp], F32, tag="acc", name="acc"),
                    }
                    stats.append(st)

                import collections
                pend = collections.deque()  # (k, pt_c) with deep PV lag

                def do_scores(k):
                    ch, it = divmod(k, IT)
                    st = stats[it]
                    ps = sps.tile([P, chunk], F32, tag="s", name="ps")
                    for ct in range(CT):
                        for nn in range(NN):
                            nc.tensor.matmul(
                                ps[:, nn * W512:(nn + 1) * W512],
                                qt_s[:, ct, it * P:(it + 1) * P],
                                (kt_loc[:, ct, nn * W512:(nn + 1) * W512]
                                 if ch == 0 else
                                 kt_full[
                                     :, ct,
                                     ch * chunk + nn * W512:
                                     ch * chunk + (nn + 1) * W512,
                                 ]),
                                start=(ct == 0),
                                stop=(ct == CT - 1),
                                skip_group_check=True,
                            )
                    nc.vector.reduce_max(
                        st["mneg"][:, ch:ch + 1], ps[:],
                        axis=mybir.AxisListType.X, negate=True,
                    )
                    if ch > 0:
                        nc.vector.tensor_tensor(
                            st["nmr"][:, ch:ch + 1], st["nmr"][:, ch - 1:ch],
                            st["mneg"][:, ch:ch + 1], op=mybir.AluOpType.min,
                        )
                    else:
                        nc.vector.tensor_copy(st["nmr"][:, :1], st["mneg"][:, :1])
                    p_c = cp.tile([P, chunk], BF16, tag="p", name="p_c", bufs=16)
                    # bias = running max -> P is exp(s - m_run(ch)); no
                    # beta, and no accum_out: the denominator comes from
                    # the PV ones-column
                    nc.scalar.activation(
                        p_c[:], ps[:],
                        mybir.ActivationFunctionType.Exp,
                        bias=st["nmr"][:, ch:ch + 1],
                        scale=1.0,
                    )
                    if ch < 2:
                        # xbar transposes are serialized against in-flight
                        # collectives; ch0/ch1 (which overlap the K and V
                        # AllGathers) must transpose on the PE instead
                        return p_c
                    pt_c = cp.tile([P, JPC, P], BF16, tag="pt", name="pt_c", bufs=30)
                    # DMA_TRANSPOSE burns ~1.3us of ISSUING-ENGINE time;
                    # keep them all on the otherwise-idle sync engine
                    nc.sync.dma_start_transpose(pt_c[:], p_c[:])
                    return pt_c

                def do_tr_pe(p_c_t):
                    # P^T via PE for ch0 (PSUM is full: borrow a scores-
                    # ring tile and pack the 8 bf16 transpose outputs into
                    # its first half via bitcast slices)
                    pt_c = cp.tile([P, JPC, P], BF16, tag="pt", name="pt_c", bufs=30)
                    ps_tr = sps.tile([P, chunk], F32, tag="s", name="ps_tr")
                    for j2 in range(JPC):
                        tpp = ps_tr[:, j2 * 64:(j2 + 1) * 64].bitcast(BF16)
                        nc.tensor.transpose(
                            tpp, p_c_t[:, j2 * P:(j2 + 1) * P], ident_b[:]
                        )
                        if j2 % 2:
                            nc.vector.tensor_copy(pt_c[:, j2, :], tpp)
                        else:
                            nc.scalar.copy(pt_c[:, j2, :], tpp)
                    return pt_c

                def do_pv(k, pt_c):
                    ch, it = divmod(k, IT)
                    st = stats[it]
                    po = ops.tile([P, dvp], F32, tag="po", name="po")
                    for j2 in range(JPC):
                        nc.tensor.matmul(
                            po[:], pt_c[:, j2, :],
                            (v_loc[:, j2, :] if ch == 0 else
                             v_s[:, ch * JPC + j2, :]),
                            start=(j2 == 0), stop=(j2 == JPC - 1),
                        )
                    if ch == 0:
                        nc.vector.tensor_copy(st["acc"][:], po[:])
                    else:
                        # gamma = exp(m_run(ch-1) - m_run(ch))
                        nc.scalar.activation(
                            st["gam"][:, ch:ch + 1], st["nmr"][:, ch - 1:ch],
                            mybir.ActivationFunctionType.Exp,
                            bias=st["nmr"][:, ch:ch + 1], scale=-1.0,
                        )
                        # acc = acc*gamma + po (P already carries beta via
                        # the running-max exp bias)
                        nc.vector.scalar_tensor_tensor(
                            st["acc"][:], st["acc"][:], st["gam"][:, ch:ch + 1],
                            po[:],
                            op0=mybir.AluOpType.mult,
                            op1=mybir.AluOpType.add,
                        )
                    if ch == NCH - 1:
                        # denominator rode along in the ones-column
                        nc.vector.reciprocal(
                            st["rinv"][:], st["acc"][:, dv:dvp]
                        )
                        nc.vector.tensor_scalar_mul(
                            st["acc"][:, :dv], st["acc"][:, :dv], st["rinv"][:]
                        )
                        nc.sync.dma_start(
                            out_ext.ap().rearrange("(it p) c -> p it c", p=P)[
                                :, it, :
                            ],
                            st["acc"][:, :dv],
                        )

                order = [g * IT + i2 for g in range(NCH) for i2 in range(IT)]
                LAG = min(28, max(1, len(order) - 1))
                LAG0 = 3  # shallow lag inside ch0 so it completes in-AG
                anchor = ma.tile([1, 64], U16, tag="anchor", name="anchor",
                                 bufs=2)

                # NOTE: Tile rotates hardware DMAs over 8 completion
                # semaphores in EMISSION order; a DMA must wait for its
                # lane's previous occupant. AG-gated assembly DMAs must
                # therefore be emitted AFTER all of ch0's transposes, or
                # ch0 (which is AG-independent) transitively waits on the
                # collective.
                TRPE = 2 * IT  # chunks with PE transposes (AG overlap)
                sc_pend = collections.deque()  # (k, p_c) awaiting PE tr
                for k in order:
                    if k == IT:
                        # ch0 fully emitted; flush its transposes + PVs
                        # ahead of the first AG1-dependent scores (ch1
                        # PVs stay queued: they need the V AllGather)
                        while sc_pend:
                            kk, pc_t = sc_pend.popleft()
                            pend.append((kk, do_tr_pe(pc_t)))
                        while pend:
                            kk, pt = pend.popleft()
                            do_pv(kk, pt)
                        # kt assembly first (scores ch1 needs slot 1
                        # before PV needs V). anchor = static-offset
                        # pack_ag read carrying the collective wait for
                        # the dynamic-offset DMAs, which Tile can't track.
                        # tile_wait_until keeps the scheduler from
                        # hoisting these AG-gated DMAs ahead of ch0's
                        # transposes/exps in the engine streams (which
                        # parks those engines on the collective).
                        # everything on the SYNC engine: it is idle at
                        # AG1-completion (exps/evictions keep scalar busy
                        # for ~30us more), so assembly starts immediately
                        with tc.tile_wait_until(1.0):
                            nc.sync.dma_start(
                                anchor[:1, :], pack_k_ag[:][1:2, 0:64]
                            )
                            rk_sync = nc.sync.cc_rank(groups)
                            for s in range(1, NCH):
                                assemble_kt(s, rk_sync)
                            nc.sync.dma_start(
                                anchor[:1, :], pack_v_ag[:][0:1, 0:64]
                            )
                            for s in range(1, NCH):
                                assemble_v(s, rk_sync)
                    if k == TRPE:
                        # last PE-transposed chunk flushes before the
                        # first xbar-transposed one
                        while sc_pend:
                            kk, pc_t = sc_pend.popleft()
                            pend.append((kk, do_tr_pe(pc_t)))
                    res = do_scores(k)
                    if k < TRPE:
                        # PE-transpose lags scores by one chunk so the
                        # in-order PE never stalls on the exp chain
                        sc_pend.append((k, res))
                        if len(sc_pend) > 1:
                            kk, pc_t = sc_pend.popleft()
                            pend.append((kk, do_tr_pe(pc_t)))
                    else:
                        pend.append((k, res))
                    lag = LAG0 if k < IT else LAG
                    if len(pend) > lag:
                        kk, pt = pend.popleft()
                        do_pv(kk, pt)
                    # once every PV's V-slot is safely assembled (AG2 done
                    # well before ch5), amortize the deep lag down so the
                    # final drain isn't ~LAG serial PVs on the PE
                    if k >= 5 * IT and len(pend) > 10:
                        kk, pt = pend.popleft()
                        do_pv(kk, pt)
                while pend:
                    kk, pt = pend.popleft()
                    do_pv(kk, pt)

            phb_cm.__exit__(None, None, None)

    nc.finalize()
    return nc


_NC_CACHE = {}


def _get_nc(key):
    if key not in _NC_CACHE:
        n, d, dqk, dv, ncores = key
        _NC_CACHE[key] = build(n=n, d=d, dqk=dqk, dv=dv, ncores=ncores)
    return _NC_CACHE[key]


def run(x, Wq, Wk, Wv, trace=False):
    n, d = x.shape
    dqk = Wq.shape[0]
    dv = Wv.shape[0]
    ncores = N_CORES
    nsh = n // ncores
    nc = _get_nc((n, d, dqk, dv, ncores))

    x = np.ascontiguousarray(x, dtype=np.float32)
    Wq = np.ascontiguousarray(Wq, dtype=np.float32)
    Wk = np.ascontiguousarray(Wk, dtype=np.float32)
    Wv = np.ascontiguousarray(Wv, dtype=np.float32)

    in_maps = [
        {"x": x[r * nsh:(r + 1) * nsh], "Wq": Wq, "Wk": Wk, "Wv": Wv}
        for r in range(ncores)
    ]
    res = run_bass_kernel_spmd(
        nc, in_maps, core_ids=list(range(ncores)), trace=trace
    )
    out = np.concatenate([res.results[r]["out"] for r in range(ncores)], axis=0)
    return out, res


def kernel(x, Wq, Wk, Wv):
    out, _ = run(x, Wq, Wk, Wv)
    return out
